# revision 20
# baseline (speedup 1.0000x reference)
"""Distributed attention forward kernel for one TRN2 chip (8 NeuronCores).

Problem: B=4, L=2048, D_IN=1024, 16 heads x 64 dim.
  qk = (x @ Wqk + bqk) / 32            -> q,k per head
  v  = (x @ Wv + bv) / 32
  out = softmax(q k^T / 64) v          -> [B, L, 1024]

Sharding: core c handles batch c//2 and heads 8*(c%2) .. +8
(data parallel over batch x tensor parallel over heads). No collectives;
the host scatters inputs and gathers the per-core [2048, 512] outputs.

Per-core dataflow (all on one NeuronCore, Tile-scheduled):
  1. DMA x rows, PE-transpose to x^T [d_in, pos] (f32).
  2. v = x @ Wv' in natural [pos, cols] layout; store as bf16 "vext" tiles
     with a fused ones-column per head ([v_h | 1]), so the attention AV
     matmul also produces the softmax denominator for free.
  3. qk^T = Wqk'^T x^T in transposed [cols, pos] layout. Host permutes
     Wqk columns so heads come in pairs: qT2[p] holds q^T of heads
     (2p, 2p+1) stacked on partitions 0-63 / 64-127, kT2[p] likewise
     (keeps matmul lhsT/rhs base partitions equal).
  4. Per head, per 1024-wide q block: for each 128-wide k chunk:
     S^T = matmul(lhsT=k^T chunk, rhs=q^T)  [128 k, 1024 q] (f32r)
     E = exp(S^T / 64) on ScalarE -> bf16
     psum_O += matmul(lhsT=vext chunk [128,65], rhs=E)  (bf16)
     Software-pipelined (AV lags S/exp by one chunk) so ScalarE's exp
     overlaps TensorE.
  5. psum_O [65, q] -> SBUF -> PE-transpose [q,65] -> row 64 is the
     softmax denominator: reciprocal + per-partition scalar multiply,
     DMA out.
"""

import sys

if "/opt/trn_rl_repo" not in sys.path:
    sys.path.insert(0, "/opt/trn_rl_repo")

from contextlib import ExitStack

import ml_dtypes
import numpy as np

import concourse.bass as bass
import concourse.mybir as mybir
from concourse import bacc
from concourse.tile import TileContext

# Problem constants (hardcoded; kernel.py must be self-contained).
B = 4
L = 2048
D_IN = 1024
HEADS = 16
DIM = 64
N_CORES = 8

H_LOC = 8          # heads per core
PAIRS = 4          # head pairs per core
QK_COLS = 1024     # 8 heads * 128 (q+k) columns per core
V_COLS = 512       # 8 heads * 64
VE_COLS = H_LOC * (DIM + 1)  # 520, v plus ones column per head
HALF = L // 2      # positions processed per projection half

F32 = mybir.dt.float32
F32R = mybir.dt.float32r
BF16 = mybir.dt.bfloat16


def build_nc():
    nc = bacc.Bacc()

    x_e = nc.declare_dram_parameter("x", [L, D_IN], BF16, isOutput=False)
    wqk_e = nc.declare_dram_parameter("wqk", [D_IN, QK_COLS], BF16, isOutput=False)
    bqk_e = nc.declare_dram_parameter("bqk2", [128, 8], F32, isOutput=False)
    wv_e = nc.declare_dram_parameter("wv", [D_IN, V_COLS], BF16, isOutput=False)
    bve_e = nc.declare_dram_parameter("bve", [128, VE_COLS], F32, isOutput=False)
    id_e = nc.declare_dram_parameter("ident", [128, 128], F32, isOutput=False)
    out_e = nc.declare_dram_parameter("out", [L, V_COLS], F32, isOutput=True)

    with TileContext(nc) as tc, ExitStack() as ctx:
        singles = ctx.enter_context(tc.tile_pool(name="singles", bufs=1))
        p_xt = ctx.enter_context(tc.tile_pool(name="xt", bufs=8))
        p_wqk = ctx.enter_context(tc.tile_pool(name="wqkp", bufs=8))
        p_wv = ctx.enter_context(tc.tile_pool(name="wvp", bufs=8))
        p_qkt = ctx.enter_context(tc.tile_pool(name="qkt", bufs=8))
        p_vext = ctx.enter_context(tc.tile_pool(name="vext", bufs=16))
        p_e = ctx.enter_context(tc.tile_pool(name="epool", bufs=6))
        p_otsb = ctx.enter_context(tc.tile_pool(name="otsb", bufs=2))
        p_outt = ctx.enter_context(tc.tile_pool(name="outt", bufs=2))
        p_rec = ctx.enter_context(tc.tile_pool(name="rec", bufs=4))
        pp_a = ctx.enter_context(tc.tile_pool(name="ppa", bufs=3, space="PSUM"))
        pp_ot = ctx.enter_context(tc.tile_pool(name="ppot", bufs=1, space="PSUM"))

        # x^T via the DMA transpose crossbar (bf16): one DMA per 128-wide
        # d_in chunk replaces PE transposes entirely. Issued first: the
        # whole projection chain waits on these.
        xt = []
        for dc in range(8):
            t = p_xt.tile([128, L], BF16, name=f"xt{dc}", tag="xt")
            nc.sync.dma_start(
                out=t, in_=x_e[:, dc * 128 : (dc + 1) * 128], transpose=True
            )
            xt.append(t)

        ident = singles.tile([128, 128], F32)
        nc.sync.dma_start(out=ident, in_=id_e[:, :])
        bqk_sb = singles.tile([128, 8], F32)
        nc.sync.dma_start(out=bqk_sb, in_=bqk_e[:, :])
        bve_sb = singles.tile([128, VE_COLS], F32)
        nc.sync.dma_start(out=bve_sb, in_=bve_e[:, :])

        # Whole wv resident: moving operand of the v projection.
        wv_t = []
        for kc in range(8):
            w = p_wv.tile([128, V_COLS], BF16, name=f"wv{kc}", tag="wv")
            nc.sync.dma_start(out=w, in_=wv_e[kc * 128 : (kc + 1) * 128, :])
            wv_t.append(w)

        # Whole wqk resident as [128, kc, 128] tiles (one 3D-AP DMA each).
        wqk_t = []
        for c in range(8):
            w = p_wqk.tile([128, 8, 128], BF16, name=f"wqk{c}", tag="wqk")
            nc.sync.dma_start(
                out=w,
                in_=wqk_e.ap()
                .rearrange("(kc p) q -> p kc q", p=128)[
                    :, :, c * 128 : (c + 1) * 128
                ],
            )
            wqk_t.append(w)

        # qk^T output tiles: chunk 2p = q^T of pair p, chunk 2p+1 = k^T.
        qk_t = [
            p_qkt.tile([128, L], BF16, name=f"qkt{c}", tag="qkt") for c in range(8)
        ]
        # v (+ ones col) tiles, one per 128-position chunk, bf16.
        ve_t = [
            p_vext.tile([128, VE_COLS], BF16, name=f"ve{i}", tag="ve")
            for i in range(16)
        ]

        def project_v_chunk(pc):
            psv = pp_a.tile([128, V_COLS], F32, tag="ps")
            for kc in range(8):
                nc.tensor.matmul(
                    psv,
                    xt[kc][:, pc * 128 : pc * 128 + 128],
                    wv_t[kc],
                    start=(kc == 0),
                    stop=(kc == 7),
                )
            ve = ve_t[pc]
            # v + bias into the per-head 64-col slots (bf16), ones into col 64.
            nc.vector.tensor_tensor(
                ve.rearrange("p (h d) -> p h d", h=H_LOC)[:, :, 0:DIM],
                psv.rearrange("p (h d) -> p h d", h=H_LOC),
                bve_sb.rearrange("p (h d) -> p h d", h=H_LOC)[:, :, 0:DIM],
                mybir.AluOpType.add,
            )
            nc.vector.tensor_copy(
                ve.rearrange("p (h d) -> p h d", h=H_LOC)[:, :, DIM : DIM + 1],
                bve_sb.rearrange("p (h d) -> p h d", h=H_LOC)[:, :, DIM : DIM + 1],
            )

        def project_qk_chunk(c):
            # chunk c of the permuted Wqk -> qk_t[c], all positions.
            for pc2 in range(4):
                psq = pp_a.tile([128, 512], F32, tag="ps")
                for kc in range(8):
                    nc.tensor.matmul(
                        psq,
                        wqk_t[c][:, kc, :],
                        xt[kc][:, pc2 * 512 : pc2 * 512 + 512],
                        start=(kc == 0),
                        stop=(kc == 7),
                    )
                nc.vector.tensor_scalar_add(
                    qk_t[c][:, pc2 * 512 : pc2 * 512 + 512],
                    psq,
                    bqk_sb[:, c : c + 1],
                )

        # Projection work is drip-fed between attention chunks: the PE
        # array otherwise idles ~25% in exp-bound stretches and the HAM
        # activity monitor halves its clock. Once real projection pieces
        # run out (last head pair), discarded projection matmuls keep the
        # array warm at zero correctness risk.
        piece_queue = [(1, 2, False), (1, 3, False)]
        for c in range(2, 8):
            for pc2 in range(4):
                piece_queue.append((c, pc2, False))
        while len(piece_queue) < 32:
            n = len(piece_queue)
            piece_queue.append((2 + n % 6, n % 4, True))

        def emit_piece():
            if not piece_queue:
                return
            c, pc2, dummy = piece_queue.pop(0)
            psq = pp_a.tile([128, 512], F32, tag="ps")
            for kc in range(8):
                nc.tensor.matmul(
                    psq,
                    wqk_t[c][:, kc, :],
                    xt[kc][:, pc2 * 512 : pc2 * 512 + 512],
                    start=(kc == 0),
                    stop=(kc == 7),
                )
            if not dummy:
                nc.vector.tensor_scalar_add(
                    qk_t[c][:, pc2 * 512 : pc2 * 512 + 512],
                    psq,
                    bqk_sb[:, c : c + 1],
                )

        def attention_head(p, hh, qh, v_jit=False):
            """Head lh=2p+hh, q block qh (1024 wide)."""
            lh = 2 * p + hh
            qt, kt = qk_t[2 * p], qk_t[2 * p + 1]
            prow = slice(hh * 64, hh * 64 + 64)
            q0 = qh * 1024
            ps_ot = pp_ot.tile([65, 1024], F32, tag="ot")
            e_tiles = []
            for kc in range(16):
                if v_jit:
                    # First head: produce v for chunk kc just before use.
                    project_v_chunk(kc)
                    if kc in (7, 12):
                        emit_piece()
                elif kc in (5, 12):
                    emit_piece()
                ps = pp_a.tile([128, 1024], F32, tag="ps")
                for i in range(2):
                    nc.tensor.matmul(
                        ps[:, i * 512 : i * 512 + 512],
                        kt[prow, kc * 128 : kc * 128 + 128],
                        qt[prow, q0 + i * 512 : q0 + i * 512 + 512],
                        start=True,
                        stop=True,
                    )
                et = p_e.tile([128, 1024], BF16, tag="E")
                nc.scalar.activation(
                    et, ps, mybir.ActivationFunctionType.Exp, scale=1.0 / 64.0
                )
                e_tiles.append(et)
                # AV lags by TWO chunks: the in-order PE queue then never
                # stalls on a recent exp (S[kc] issues ahead of AV[kc-2]).
                if kc > 1:
                    _av(ps_ot, ve_t[kc - 2], e_tiles[kc - 2], lh, kc - 2)
            _av(ps_ot, ve_t[14], e_tiles[14], lh, 14)
            _av(ps_ot, ve_t[15], e_tiles[15], lh, 15)

            # Finale: transpose 65 x q -> q x 65, divide by denominator row.
            ot_sb = p_otsb.tile([65, 1024], F32, tag="ot_sb")
            nc.vector.tensor_copy(ot_sb, ps_ot)
            ott = p_outt.tile([128, 8, DIM], F32, tag="ott")
            for qc in range(8):
                ptr = pp_a.tile([128, 65], F32, tag="ps")
                nc.tensor.transpose(
                    ptr, ot_sb[:, qc * 128 : qc * 128 + 128], ident[0:65, 0:65]
                )
                rec = p_rec.tile([128, 1], F32, tag="rec")
                nc.vector.reciprocal(rec, ptr[:, 64:65])
                nc.vector.tensor_scalar_mul(ott[:, qc, :], ptr[:, 0:DIM], rec)
            # One batched output DMA per (head, q block): sync-engine issue
            # cost is ~0.6 us per DMA instruction, so batch 8 chunks.
            nc.sync.dma_start(
                out=out_e.ap().rearrange("(qq p) n -> p qq n", p=128)[
                    :, 8 * qh : 8 * qh + 8, lh * DIM : (lh + 1) * DIM
                ],
                in_=ott,
            )

        def _av(ps_ot, ve, et, lh, kc):
            for i in range(2):
                nc.tensor.matmul(
                    ps_ot[:, i * 512 : i * 512 + 512],
                    ve[:, lh * 65 : lh * 65 + 65],
                    et[:, i * 512 : i * 512 + 512],
                    start=(kc == 0),
                    stop=(kc == 15),
                )

        # Minimum upfront projection: all of chunk 0 (q side of pair 0)
        # plus the first half of chunk 1 (k side, chunks 0-7); the rest of
        # chunk 1 drips in just-in-time during the first head.
        project_qk_chunk(0)
        for pc2 in range(2):
            psq = pp_a.tile([128, 512], F32, tag="ps")
            for kc in range(8):
                nc.tensor.matmul(
                    psq,
                    wqk_t[1][:, kc, :],
                    xt[kc][:, pc2 * 512 : pc2 * 512 + 512],
                    start=(kc == 0),
                    stop=(kc == 7),
                )
            nc.vector.tensor_scalar_add(
                qk_t[1][:, pc2 * 512 : pc2 * 512 + 512],
                psq,
                bqk_sb[:, 1:2],
            )
        first = True
        for p in range(PAIRS):
            for hh, qh in [(0, 0), (0, 1), (1, 0), (1, 1)]:
                attention_head(p, hh, qh, v_jit=first)
                first = False

    nc.compile()
    return nc


def host_prep(x, Wqk, bqk, Wv, bv, core):
    """Per-core input shard with host-folded scales and layouts."""
    b = core // 2
    base = (core % 2) * H_LOC
    s = np.float32(1.0 / 32.0)  # 1 / d_in**0.5 for both qk and v projections

    cols = []
    for p in range(PAIRS):
        g0 = base + 2 * p
        g1 = g0 + 1
        cols.extend(range(g0 * 128, g0 * 128 + 64))
        cols.extend(range(g1 * 128, g1 * 128 + 64))
        cols.extend(range(g0 * 128 + 64, g0 * 128 + 128))
        cols.extend(range(g1 * 128 + 64, g1 * 128 + 128))
    cols = np.asarray(cols)

    wqk_d = np.ascontiguousarray((Wqk[:, cols] * s).astype(ml_dtypes.bfloat16))
    bqk_d = np.ascontiguousarray(
        (bqk[cols] * s).reshape(8, 128).T, dtype=np.float32
    )
    wv_d = np.ascontiguousarray(
        (Wv[:, base * DIM : (base + H_LOC) * DIM] * s).astype(ml_dtypes.bfloat16)
    )
    bve = np.zeros((H_LOC, DIM + 1), np.float32)
    bve[:, :DIM] = (bv[base * DIM : (base + H_LOC) * DIM] * s).reshape(H_LOC, DIM)
    bve[:, DIM] = 1.0
    bve_d = np.ascontiguousarray(
        np.broadcast_to(bve.reshape(1, VE_COLS), (128, VE_COLS)), dtype=np.float32
    )
    return {
        "x": np.ascontiguousarray(x[b].astype(ml_dtypes.bfloat16)),
        "wqk": wqk_d,
        "bqk2": bqk_d,
        "wv": wv_d,
        "bve": bve_d,
        "ident": np.eye(128, dtype=np.float32),
    }


_NC_CACHE = None


def _get_nc():
    global _NC_CACHE
    if _NC_CACHE is None:
        _NC_CACHE = build_nc()
    return _NC_CACHE


def run(inputs, **spmd_kwargs):
    """Run on the 8 NeuronCores; returns (full_output, BassKernelResults)."""
    from concourse.bass_utils import run_bass_kernel_spmd

    x = np.asarray(inputs["x"], dtype=np.float32)
    wqk = np.asarray(inputs["Wqk"], dtype=np.float32)
    bqk = np.asarray(inputs["bqk"], dtype=np.float32)
    wv = np.asarray(inputs["Wv"], dtype=np.float32)
    bv = np.asarray(inputs["bv"], dtype=np.float32)

    in_maps = [host_prep(x, wqk, bqk, wv, bv, c) for c in range(N_CORES)]
    nc = _get_nc()
    res = run_bass_kernel_spmd(nc, in_maps, core_ids=list(range(N_CORES)), **spmd_kwargs)

    out = np.empty((B, L, HEADS * DIM), np.float32)
    for c in range(N_CORES):
        b = c // 2
        base = (c % 2) * H_LOC
        out[b][:, base * DIM : (base + H_LOC) * DIM] = res.results[c]["out"]
    return out, res


def kernel(**inputs):
    out, _ = run(inputs)
    return out


# revision 21
# speedup vs baseline: 1.0016x; 1.0016x over previous
"""Distributed attention forward kernel for one TRN2 chip (8 NeuronCores).

Problem: B=4, L=2048, D_IN=1024, 16 heads x 64 dim.
  qk = (x @ Wqk + bqk) / 32            -> q,k per head
  v  = (x @ Wv + bv) / 32
  out = softmax(q k^T / 64) v          -> [B, L, 1024]

Sharding: core c handles batch c//2 and heads 8*(c%2) .. +8
(data parallel over batch x tensor parallel over heads). No collectives;
the host scatters inputs and gathers the per-core [2048, 512] outputs.

Per-core dataflow (all on one NeuronCore, Tile-scheduled):
  1. DMA x rows, PE-transpose to x^T [d_in, pos] (f32).
  2. v = x @ Wv' in natural [pos, cols] layout; store as bf16 "vext" tiles
     with a fused ones-column per head ([v_h | 1]), so the attention AV
     matmul also produces the softmax denominator for free.
  3. qk^T = Wqk'^T x^T in transposed [cols, pos] layout. Host permutes
     Wqk columns so heads come in pairs: qT2[p] holds q^T of heads
     (2p, 2p+1) stacked on partitions 0-63 / 64-127, kT2[p] likewise
     (keeps matmul lhsT/rhs base partitions equal).
  4. Per head, per 1024-wide q block: for each 128-wide k chunk:
     S^T = matmul(lhsT=k^T chunk, rhs=q^T)  [128 k, 1024 q] (f32r)
     E = exp(S^T / 64) on ScalarE -> bf16
     psum_O += matmul(lhsT=vext chunk [128,65], rhs=E)  (bf16)
     Software-pipelined (AV lags S/exp by one chunk) so ScalarE's exp
     overlaps TensorE.
  5. psum_O [65, q] -> SBUF -> PE-transpose [q,65] -> row 64 is the
     softmax denominator: reciprocal + per-partition scalar multiply,
     DMA out.
"""

import sys

if "/opt/trn_rl_repo" not in sys.path:
    sys.path.insert(0, "/opt/trn_rl_repo")

from contextlib import ExitStack

import ml_dtypes
import numpy as np

import concourse.bass as bass
import concourse.mybir as mybir
from concourse import bacc
from concourse.tile import TileContext

# Problem constants (hardcoded; kernel.py must be self-contained).
B = 4
L = 2048
D_IN = 1024
HEADS = 16
DIM = 64
N_CORES = 8

H_LOC = 8          # heads per core
PAIRS = 4          # head pairs per core
QK_COLS = 1024     # 8 heads * 128 (q+k) columns per core
V_COLS = 512       # 8 heads * 64
VE_COLS = H_LOC * (DIM + 1)  # 520, v plus ones column per head
HALF = L // 2      # positions processed per projection half

F32 = mybir.dt.float32
F32R = mybir.dt.float32r
BF16 = mybir.dt.bfloat16


def build_nc():
    nc = bacc.Bacc()

    x_e = nc.declare_dram_parameter("x", [L, D_IN], BF16, isOutput=False)
    wqk_e = nc.declare_dram_parameter("wqk", [D_IN, QK_COLS], BF16, isOutput=False)
    bqk_e = nc.declare_dram_parameter("bqk2", [128, 8], F32, isOutput=False)
    wv_e = nc.declare_dram_parameter("wv", [D_IN, V_COLS], BF16, isOutput=False)
    bve_e = nc.declare_dram_parameter("bve", [128, VE_COLS], F32, isOutput=False)
    id_e = nc.declare_dram_parameter("ident", [128, 128], F32, isOutput=False)
    out_e = nc.declare_dram_parameter("out", [L, V_COLS], F32, isOutput=True)

    with TileContext(nc) as tc, ExitStack() as ctx:
        singles = ctx.enter_context(tc.tile_pool(name="singles", bufs=1))
        p_xt = ctx.enter_context(tc.tile_pool(name="xt", bufs=8))
        p_wqk = ctx.enter_context(tc.tile_pool(name="wqkp", bufs=8))
        p_wv = ctx.enter_context(tc.tile_pool(name="wvp", bufs=8))
        p_qkt = ctx.enter_context(tc.tile_pool(name="qkt", bufs=8))
        p_vext = ctx.enter_context(tc.tile_pool(name="vext", bufs=16))
        p_e = ctx.enter_context(tc.tile_pool(name="epool", bufs=6))
        p_otsb = ctx.enter_context(tc.tile_pool(name="otsb", bufs=2))
        p_outt = ctx.enter_context(tc.tile_pool(name="outt", bufs=2))
        p_rec = ctx.enter_context(tc.tile_pool(name="rec", bufs=4))
        pp_a = ctx.enter_context(tc.tile_pool(name="ppa", bufs=3, space="PSUM"))
        pp_ot = ctx.enter_context(tc.tile_pool(name="ppot", bufs=1, space="PSUM"))

        # x^T via the DMA transpose crossbar (bf16): one DMA per 128-wide
        # d_in chunk replaces PE transposes entirely. Issued first: the
        # whole projection chain waits on these.
        xt = []
        for dc in range(8):
            t = p_xt.tile([128, L], BF16, name=f"xt{dc}", tag="xt")
            nc.sync.dma_start(
                out=t, in_=x_e[:, dc * 128 : (dc + 1) * 128], transpose=True
            )
            xt.append(t)

        ident = singles.tile([128, 128], F32)
        nc.sync.dma_start(out=ident, in_=id_e[:, :])
        bqk_sb = singles.tile([128, 8], F32)
        nc.sync.dma_start(out=bqk_sb, in_=bqk_e[:, :])
        bve_sb = singles.tile([128, VE_COLS], F32)
        nc.sync.dma_start(out=bve_sb, in_=bve_e[:, :])

        # Whole wv resident: moving operand of the v projection.
        wv_t = []
        for kc in range(8):
            w = p_wv.tile([128, V_COLS], BF16, name=f"wv{kc}", tag="wv")
            nc.sync.dma_start(out=w, in_=wv_e[kc * 128 : (kc + 1) * 128, :])
            wv_t.append(w)

        # Whole wqk resident as [128, kc, 128] tiles (one 3D-AP DMA each).
        wqk_t = []
        for c in range(8):
            w = p_wqk.tile([128, 8, 128], BF16, name=f"wqk{c}", tag="wqk")
            nc.sync.dma_start(
                out=w,
                in_=wqk_e.ap()
                .rearrange("(kc p) q -> p kc q", p=128)[
                    :, :, c * 128 : (c + 1) * 128
                ],
            )
            wqk_t.append(w)

        # qk^T output tiles: chunk 2p = q^T of pair p, chunk 2p+1 = k^T.
        qk_t = [
            p_qkt.tile([128, L], BF16, name=f"qkt{c}", tag="qkt") for c in range(8)
        ]
        # v (+ ones col) tiles, one per 128-position chunk, bf16.
        ve_t = [
            p_vext.tile([128, VE_COLS], BF16, name=f"ve{i}", tag="ve")
            for i in range(16)
        ]

        def project_v_chunk(pc):
            psv = pp_a.tile([128, V_COLS], F32, tag="ps")
            for kc in range(8):
                nc.tensor.matmul(
                    psv,
                    xt[kc][:, pc * 128 : pc * 128 + 128],
                    wv_t[kc],
                    start=(kc == 0),
                    stop=(kc == 7),
                )
            ve = ve_t[pc]
            # v + bias into the per-head 64-col slots (bf16), ones into col 64.
            nc.vector.tensor_tensor(
                ve.rearrange("p (h d) -> p h d", h=H_LOC)[:, :, 0:DIM],
                psv.rearrange("p (h d) -> p h d", h=H_LOC),
                bve_sb.rearrange("p (h d) -> p h d", h=H_LOC)[:, :, 0:DIM],
                mybir.AluOpType.add,
            )
            nc.vector.tensor_copy(
                ve.rearrange("p (h d) -> p h d", h=H_LOC)[:, :, DIM : DIM + 1],
                bve_sb.rearrange("p (h d) -> p h d", h=H_LOC)[:, :, DIM : DIM + 1],
            )

        def project_qk_chunk(c):
            # chunk c of the permuted Wqk -> qk_t[c], all positions.
            for pc2 in range(4):
                psq = pp_a.tile([128, 512], F32, tag="ps")
                for kc in range(8):
                    nc.tensor.matmul(
                        psq,
                        wqk_t[c][:, kc, :],
                        xt[kc][:, pc2 * 512 : pc2 * 512 + 512],
                        start=(kc == 0),
                        stop=(kc == 7),
                    )
                nc.vector.tensor_scalar_add(
                    qk_t[c][:, pc2 * 512 : pc2 * 512 + 512],
                    psq,
                    bqk_sb[:, c : c + 1],
                )

        # Projection work is drip-fed between attention chunks: the PE
        # array otherwise idles ~25% in exp-bound stretches and the HAM
        # activity monitor halves its clock. Once real projection pieces
        # run out (last head pair), discarded projection matmuls keep the
        # array warm at zero correctness risk.
        piece_queue = [(1, 2, False), (1, 3, False)]
        for c in range(2, 8):
            for pc2 in range(4):
                piece_queue.append((c, pc2, False))
        while len(piece_queue) < 47:
            n = len(piece_queue)
            piece_queue.append((2 + n % 6, n % 4, True))

        def emit_piece():
            if not piece_queue:
                return
            c, pc2, dummy = piece_queue.pop(0)
            psq = pp_a.tile([128, 512], F32, tag="ps")
            n_mm = 4 if dummy else 8
            for kc in range(n_mm):
                nc.tensor.matmul(
                    psq,
                    wqk_t[c][:, kc, :],
                    xt[kc][:, pc2 * 512 : pc2 * 512 + 512],
                    start=(kc == 0),
                    stop=(kc == n_mm - 1),
                )
            if not dummy:
                nc.vector.tensor_scalar_add(
                    qk_t[c][:, pc2 * 512 : pc2 * 512 + 512],
                    psq,
                    bqk_sb[:, c : c + 1],
                )

        def attention_head(p, hh, qh, v_jit=False):
            """Head lh=2p+hh, q block qh (1024 wide)."""
            lh = 2 * p + hh
            qt, kt = qk_t[2 * p], qk_t[2 * p + 1]
            prow = slice(hh * 64, hh * 64 + 64)
            q0 = qh * 1024
            ps_ot = pp_ot.tile([65, 1024], F32, tag="ot")
            e_tiles = []
            for kc in range(16):
                if v_jit:
                    # First head: produce v for chunk kc just before use.
                    project_v_chunk(kc)
                    if kc in (7, 12):
                        emit_piece()
                elif kc in (2, 7, 12):
                    emit_piece()
                ps = pp_a.tile([128, 1024], F32, tag="ps")
                for i in range(2):
                    nc.tensor.matmul(
                        ps[:, i * 512 : i * 512 + 512],
                        kt[prow, kc * 128 : kc * 128 + 128],
                        qt[prow, q0 + i * 512 : q0 + i * 512 + 512],
                        start=True,
                        stop=True,
                    )
                et = p_e.tile([128, 1024], BF16, tag="E")
                nc.scalar.activation(
                    et, ps, mybir.ActivationFunctionType.Exp, scale=1.0 / 64.0
                )
                e_tiles.append(et)
                # AV lags by TWO chunks: the in-order PE queue then never
                # stalls on a recent exp (S[kc] issues ahead of AV[kc-2]).
                if kc > 1:
                    _av(ps_ot, ve_t[kc - 2], e_tiles[kc - 2], lh, kc - 2)
            _av(ps_ot, ve_t[14], e_tiles[14], lh, 14)
            _av(ps_ot, ve_t[15], e_tiles[15], lh, 15)

            # Finale: transpose 65 x q -> q x 65, divide by denominator row.
            ot_sb = p_otsb.tile([65, 1024], F32, tag="ot_sb")
            nc.vector.tensor_copy(ot_sb, ps_ot)
            ott = p_outt.tile([128, 8, DIM], F32, tag="ott")
            for qc in range(8):
                ptr = pp_a.tile([128, 65], F32, tag="ps")
                nc.tensor.transpose(
                    ptr, ot_sb[:, qc * 128 : qc * 128 + 128], ident[0:65, 0:65]
                )
                rec = p_rec.tile([128, 1], F32, tag="rec")
                nc.vector.reciprocal(rec, ptr[:, 64:65])
                nc.vector.tensor_scalar_mul(ott[:, qc, :], ptr[:, 0:DIM], rec)
            # One batched output DMA per (head, q block): sync-engine issue
            # cost is ~0.6 us per DMA instruction, so batch 8 chunks.
            nc.sync.dma_start(
                out=out_e.ap().rearrange("(qq p) n -> p qq n", p=128)[
                    :, 8 * qh : 8 * qh + 8, lh * DIM : (lh + 1) * DIM
                ],
                in_=ott,
            )

        def _av(ps_ot, ve, et, lh, kc):
            for i in range(2):
                nc.tensor.matmul(
                    ps_ot[:, i * 512 : i * 512 + 512],
                    ve[:, lh * 65 : lh * 65 + 65],
                    et[:, i * 512 : i * 512 + 512],
                    start=(kc == 0),
                    stop=(kc == 15),
                )

        # Minimum upfront projection: all of chunk 0 (q side of pair 0)
        # plus the first half of chunk 1 (k side, chunks 0-7); the rest of
        # chunk 1 drips in just-in-time during the first head.
        project_qk_chunk(0)
        for pc2 in range(2):
            psq = pp_a.tile([128, 512], F32, tag="ps")
            for kc in range(8):
                nc.tensor.matmul(
                    psq,
                    wqk_t[1][:, kc, :],
                    xt[kc][:, pc2 * 512 : pc2 * 512 + 512],
                    start=(kc == 0),
                    stop=(kc == 7),
                )
            nc.vector.tensor_scalar_add(
                qk_t[1][:, pc2 * 512 : pc2 * 512 + 512],
                psq,
                bqk_sb[:, 1:2],
            )
        first = True
        for p in range(PAIRS):
            for hh, qh in [(0, 0), (0, 1), (1, 0), (1, 1)]:
                attention_head(p, hh, qh, v_jit=first)
                first = False

    nc.compile()
    return nc


def host_prep(x, Wqk, bqk, Wv, bv, core):
    """Per-core input shard with host-folded scales and layouts."""
    b = core // 2
    base = (core % 2) * H_LOC
    s = np.float32(1.0 / 32.0)  # 1 / d_in**0.5 for both qk and v projections

    cols = []
    for p in range(PAIRS):
        g0 = base + 2 * p
        g1 = g0 + 1
        cols.extend(range(g0 * 128, g0 * 128 + 64))
        cols.extend(range(g1 * 128, g1 * 128 + 64))
        cols.extend(range(g0 * 128 + 64, g0 * 128 + 128))
        cols.extend(range(g1 * 128 + 64, g1 * 128 + 128))
    cols = np.asarray(cols)

    wqk_d = np.ascontiguousarray((Wqk[:, cols] * s).astype(ml_dtypes.bfloat16))
    bqk_d = np.ascontiguousarray(
        (bqk[cols] * s).reshape(8, 128).T, dtype=np.float32
    )
    wv_d = np.ascontiguousarray(
        (Wv[:, base * DIM : (base + H_LOC) * DIM] * s).astype(ml_dtypes.bfloat16)
    )
    bve = np.zeros((H_LOC, DIM + 1), np.float32)
    bve[:, :DIM] = (bv[base * DIM : (base + H_LOC) * DIM] * s).reshape(H_LOC, DIM)
    bve[:, DIM] = 1.0
    bve_d = np.ascontiguousarray(
        np.broadcast_to(bve.reshape(1, VE_COLS), (128, VE_COLS)), dtype=np.float32
    )
    return {
        "x": np.ascontiguousarray(x[b].astype(ml_dtypes.bfloat16)),
        "wqk": wqk_d,
        "bqk2": bqk_d,
        "wv": wv_d,
        "bve": bve_d,
        "ident": np.eye(128, dtype=np.float32),
    }


_NC_CACHE = None


def _get_nc():
    global _NC_CACHE
    if _NC_CACHE is None:
        _NC_CACHE = build_nc()
    return _NC_CACHE


def run(inputs, **spmd_kwargs):
    """Run on the 8 NeuronCores; returns (full_output, BassKernelResults)."""
    from concourse.bass_utils import run_bass_kernel_spmd

    x = np.asarray(inputs["x"], dtype=np.float32)
    wqk = np.asarray(inputs["Wqk"], dtype=np.float32)
    bqk = np.asarray(inputs["bqk"], dtype=np.float32)
    wv = np.asarray(inputs["Wv"], dtype=np.float32)
    bv = np.asarray(inputs["bv"], dtype=np.float32)

    in_maps = [host_prep(x, wqk, bqk, wv, bv, c) for c in range(N_CORES)]
    nc = _get_nc()
    res = run_bass_kernel_spmd(nc, in_maps, core_ids=list(range(N_CORES)), **spmd_kwargs)

    out = np.empty((B, L, HEADS * DIM), np.float32)
    for c in range(N_CORES):
        b = c // 2
        base = (c % 2) * H_LOC
        out[b][:, base * DIM : (base + H_LOC) * DIM] = res.results[c]["out"]
    return out, res


def kernel(**inputs):
    out, _ = run(inputs)
    return out
